# revision 18
# baseline (speedup 1.0000x reference)
"""GAU-style module (InstanceNorm + gated spatial-softmax attention) on 8 trn2 cores.

Math notes (vs the PyTorch/JAX reference):
- 2D RoPE rotates q and k by the SAME per-(pair,pixel) angle, and sim is the
  per-pixel dot product, so the rotations cancel exactly:
      sim = q.k = sum_c (g0*z+b0)(g1*z+b1)
          = sum_c a_c z_c^2 + bv_c z_c + const,  a = g0*g1, bv = g0*b1+g1*b0
- The linear term is linear in x_n:  sum_c bv_c z_c = (z_W^T bv) . x_n + bv.z_b
  so z itself is only needed squared; Square() is fused into the PSUM read.
- Softmax over 4096 pixels/sample skips the max subtraction (|sim| stays far
  below fp32 exp overflow for this problem family; verified in testing).

Sharding: pure data parallel, 16 samples -> 2 per core, params replicated.

Implementation notes:
- All matmul operands are bf16 (1 cycle/row on the PE); PSUM accumulates fp32.
- Per-pixel softmax weights are applied via a rank-1 ones-matmul broadcast.
- (v + vb) * g is fused into one DVE scalar_tensor_tensor reading PSUM.
- dma_start dispatch costs ~0.65us of sequencer time each, so constants are
  packed into three tensors and output stores are batched per [128, 2048].
"""

import numpy as np

import concourse.bass as bass
import concourse.tile as tile
from concourse import mybir
from concourse.bass_utils import run_bass_kernel_spmd

F32 = mybir.dt.float32
BF16 = mybir.dt.bfloat16
AF = mybir.ActivationFunctionType
ALU = mybir.AluOpType

B, C, H, W, S, O = 16, 256, 64, 64, 128, 256
P = H * W            # 4096 pixels per sample
NCORES = 8
SPC = B // NCORES    # samples per core
NCHUNK = 8           # pixel chunks per sample
CW = P // NCHUNK     # chunk width = 512 (one fp32 PSUM bank)
HALF = P // 2
EPS = 1e-5

_cache: dict = {}

# The walrus build in this container refuses instructions carrying more than
# one semaphore wait ("Too many sync wait commands").  Tile freely emits 3-5
# waits per instruction (and ~27 on the tail drain).  Since engine queues
# execute in order, a wait carried by a same-engine NoOp immediately before
# the instruction is semantically identical — so split excess waits onto NoOp
# carriers at instruction-commit time.
MAXW = 1
_patched = False


def _install_tile_patches():
    global _patched
    if _patched:
        return
    _patched = True
    from concourse.vector_clock import ScopedClock

    orig_add = tile.TileContext._add_instruction

    def patched_add(self, inst):
        si = inst.sync_info
        if si is not None and si.on_wait and len(si.on_wait) > MAXW:
            waits = list(si.on_wait)
            excess, keep = waits[:-MAXW], waits[-MAXW:]
            for j, w in enumerate(excess):
                nop = mybir.InstNoOp(name=f"{inst.name}_ws{j}", ins=[], outs=[])
                nop.engine = inst.engine
                nop.sync_info = mybir.SyncInfo(on_wait=[w], on_update=[])
                orig_add(self, nop)
            inst.sync_info = mybir.SyncInfo(on_wait=keep, on_update=si.on_update or [])
        orig_add(self, inst)

    tile.TileContext._add_instruction = patched_add

    def patched_dab(self, tick_clock, wait_clock):
        drain_inst = self.nc.sync.drain()
        wait_clock.add_sem_waits(
            drain_inst.ins, ScopedClock({None: tick_clock.global_clock})
        )
        mi = drain_inst.ins
        si = mi.sync_info
        if si is not None and si.on_wait and len(si.on_wait) > MAXW:
            waits = list(si.on_wait)
            keep, rest = waits[:MAXW], waits[MAXW:]
            mi.sync_info = mybir.SyncInfo(on_wait=keep, on_update=si.on_update or [])
            for w in rest:
                n = self.nc.sync.nop()
                n.ins.sync_info = mybir.SyncInfo(on_wait=[w], on_update=[])
        self.nc.all_engine_barrier()
        popped = self.nc._tile_sem_poison_stack.pop()
        assert popped is self._sem_poison
        self.nc.clear_and_free_semaphores(list(self.sems.allocated().values()))
        self.nc.all_engine_barrier()

    tile.TileContext._drain_and_barrier = patched_dab


def _build():
    _install_tile_patches()
    nc = bass.Bass("TRN2", target_bir_lowering=False, debug=False)

    x_d = nc.dram_tensor("x", [SPC, C, P], F32, kind="ExternalInput")
    # bf16 const pack: [wz(2x128) | ohA(64) | ohBW(2x64)] = 448 cols
    cbf_d = nc.dram_tensor("cbf", [128, 448], BF16, kind="ExternalInput")
    # f32 const pack: [zb | gb0 gb1 | vb0 vb1 | pb0 pb1 | c0] = 8 cols
    cf32_d = nc.dram_tensor("cf32", [128, 8], F32, kind="ExternalInput")
    # big weights: [wg(2x256) | wv(2x256) | wp(2x256)] along the last dim
    wbig_d = nc.dram_tensor("wbig", [128, 2, 768], BF16, kind="ExternalInput")

    out_d = nc.dram_tensor("out", [SPC, O, P], F32, kind="ExternalOutput")
    a_d = nc.dram_tensor("A", [SPC, P], F32, kind="ExternalOutput")

    with tile.TileContext(nc) as tc:
        with (
            tc.tile_pool(name="consts", bufs=1) as consts,
            tc.tile_pool(name="xp", bufs=2) as xp,
            tc.tile_pool(name="xnp", bufs=4) as xnp,
            tc.tile_pool(name="gbf", bufs=2) as gbfp,
            tc.tile_pool(name="ybf", bufs=2) as ybfp,
            tc.tile_pool(name="abcp", bufs=2) as abcp,
            tc.tile_pool(name="wp", bufs=4) as wpool,
            tc.tile_pool(name="sqp", bufs=4) as sqp,
            tc.tile_pool(name="outp", bufs=3) as outp,
            tc.tile_pool(name="tiny", bufs=2) as tiny,
            tc.tile_pool(name="rowp", bufs=1) as rowp,
            tc.tile_pool(name="psmm", bufs=7, space="PSUM") as psmm,
            tc.tile_pool(name="psaux", bufs=1, space="PSUM") as psaux,
        ):
            # ---- constants / weights (loaded once, few big DMAs) ----
            cbf = consts.tile([128, 448], BF16, tag="cbf")
            cf32 = consts.tile([128, 8], F32, tag="cf32")
            wbig = consts.tile([128, 2, 768], BF16, tag="wbig")
            nc.sync.dma_start(out=cbf[:], in_=cbf_d[:])
            nc.sync.dma_start(out=cf32[:], in_=cf32_d[:])

            def load_big_weights():
                nc.sync.dma_start(out=wbig[:], in_=wbig_d[:])

            def wzv(kt):
                return cbf[:, kt * 128:(kt + 1) * 128]

            def ohav(c):
                return cbf[:, 256 + c * 8:256 + (c + 1) * 8]

            def ohbv(kt, c):
                return cbf[:, 320 + kt * 64 + c * 8:320 + kt * 64 + (c + 1) * 8]

            def wgv(kt, mt):
                return wbig[:, kt, mt * 128:(mt + 1) * 128]

            def wvv(kt, mt):
                return wbig[:, kt, 256 + mt * 128:256 + (mt + 1) * 128]

            def wpv(kt, mt):
                return wbig[:, kt, 512 + mt * 128:512 + (mt + 1) * 128]

            zb = cf32[:, 0:1]
            gbv = [cf32[:, 1:2], cf32[:, 2:3]]
            vbv = [cf32[:, 3:4], cf32[:, 4:5]]
            pbv = [cf32[:, 5:6], cf32[:, 6:7]]
            c0 = cf32[0:NCHUNK, 7:8]

            ones_bf = consts.tile([1, 128], BF16, tag="ones_bf")
            nc.vector.memset(ones_bf[:], 1.0)
            ones_f = consts.tile([1, 128], F32, tag="ones_f")
            nc.vector.memset(ones_f[:], 1.0)
            ones8 = consts.tile([NCHUNK, 1], F32, tag="ones8")
            nc.vector.memset(ones8[:], 1.0)
            eps_t = consts.tile([128, 1], F32, tag="eps")
            nc.vector.memset(eps_t[:], EPS)

            def cs(c):
                return slice(c * CW, (c + 1) * CW)

            # per-sample state carried between phases
            st: list[dict] = [dict() for _ in range(SPC)]

            def phase_load_norm(s):
                d = st[s]
                xn = []
                for ct in range(2):
                    xt = xp.tile([128, P], F32, tag="x", name=f"x{ct}")
                    for q in range(2):
                        qs = slice(q * HALF, (q + 1) * HALF)
                        nc.sync.dma_start(out=xt[:, qs],
                                          in_=x_d[s, ct * 128:(ct + 1) * 128, qs])
                    stt = tiny.tile([128, NCHUNK, 6], F32, tag="bnst")
                    for j in range(NCHUNK):
                        nc.vector.bn_stats(out=stt[:, j, :], in_=xt[:, cs(j)])
                    mv = tiny.tile([128, 2], F32, tag="mv")
                    nc.vector.bn_aggr(out=mv[:], in_=stt[:])
                    sd = tiny.tile([128, 1], F32, tag="sd")
                    nc.scalar.activation(sd[:], mv[:, 1:2], AF.Sqrt, bias=eps_t[:], scale=1.0)
                    rs_ = tiny.tile([128, 1], F32, tag="rstd")
                    nc.vector.reciprocal(rs_[:], sd[:])
                    nmr = tiny.tile([128, 1], F32, tag="nmr")
                    nc.vector.tensor_scalar(nmr[:], mv[:, 0:1], rs_[:], -1.0, ALU.mult, ALU.mult)
                    xnt = xnp.tile([128, P], BF16, tag="xn", name=f"xn{ct}")
                    for h in range(2):
                        hs = slice(h * HALF, (h + 1) * HALF)
                        if ct == 0:
                            nc.gpsimd.tensor_scalar(xnt[:, hs], xt[:, hs], rs_[:], nmr[:],
                                                    ALU.mult, ALU.add)
                        else:
                            nc.scalar.activation(xnt[:, hs], xt[:, hs], AF.Identity,
                                                 bias=nmr[:], scale=rs_[:])
                    xn.append(xnt)
                d["xn"] = xn

            def phase_z(s):
                d = st[s]
                xn = d["xn"]
                # z matmuls -> fused Square -> sim accumulation ([8, CW] psum)
                psim = psaux.tile([NCHUNK, CW], F32, tag="aux")
                for half in range(2):
                    crange = range(half * 4, half * 4 + 4)
                    psz = {}
                    for kt in range(2):
                        for c in crange:
                            if kt == 0:
                                psz[c] = psmm.tile([128, CW], F32, tag="mm", name=f"psz{c}")
                            nc.tensor.matmul(psz[c][:], wzv(kt), xn[kt][:, cs(c)],
                                             start=(kt == 0), stop=(kt == 1))
                    for c in crange:
                        sq = sqp.tile([128, CW], BF16, tag="sq")
                        nc.scalar.activation(sq[:], psz[c][:], AF.Square, bias=zb, scale=1.0)
                        nc.tensor.matmul(psim[:], ohav(c), sq[:],
                                         start=(c == 0), stop=False)
                        nc.tensor.matmul(psim[:], ohbv(0, c), xn[0][:, cs(c)],
                                         start=False, stop=False)
                        nc.tensor.matmul(psim[:], ohbv(1, c), xn[1][:, cs(c)],
                                         start=False, stop=(c == NCHUNK - 1))
                d["psim"] = psim

            def phase_softmax(s):
                d = st[s]
                psim = d.pop("psim")
                # softmax over the 4096 pixels (no max subtraction needed)
                e8 = tiny.tile([NCHUNK, CW], F32, tag="e8")
                rsum = tiny.tile([NCHUNK, 1], F32, tag="rsum")
                nc.scalar.activation(e8[:], psim[:], AF.Exp, bias=c0, scale=1.0,
                                     accum_out=rsum[:])
                pss = psaux.tile([1, 1], F32, tag="aux")
                nc.tensor.matmul(pss[:], ones8[:], rsum[:], start=True, stop=True)
                s_sb = tiny.tile([1, 1], F32, tag="ssb")
                nc.vector.tensor_copy(s_sb[:], pss[:])
                psbc = psaux.tile([128, 1], F32, tag="aux")
                nc.tensor.matmul(psbc[:], ones_f[:], s_sb[:], start=True, stop=True)
                rs_all = tiny.tile([128, 1], F32, tag="rsall")
                nc.vector.reciprocal(rs_all[:], psbc[:])
                a8 = tiny.tile([NCHUNK, CW], F32, tag="a8")
                nc.vector.tensor_scalar_mul(a8[:], e8[:], rs_all[0:NCHUNK, :])
                nc.gpsimd.dma_start(out=a_d[s].rearrange("(p f) -> p f", p=NCHUNK), in_=a8[:])
                a8b = tiny.tile([NCHUNK, CW], BF16, tag="a8b")
                nc.vector.tensor_copy(a8b[:], a8[:])
                # gather the 8 partition-rows into one [1, P] row so the
                # broadcast matmul's moving operand starts at partition 0
                a8row = rowp.tile([1, P], BF16, tag="a8row")
                nc.sync.dma_start(
                    out=a8row[0:1, :].rearrange("p (c f) -> p c f", c=NCHUNK),
                    in_=a8b[:, :])
                d["a8row"] = a8row

            def phase_gvw(s):
                d = st[s]
                xn, a8row = d["xn"], d["a8row"]
                g_bf = [gbfp.tile([128, P], BF16, tag="g", name=f"g{i}") for i in range(2)]
                y_bf = [ybfp.tile([128, P], BF16, tag="y", name=f"y{i}") for i in range(2)]
                abc = abcp.tile([128, P], BF16, tag="abc")
                wt = [wpool.tile([128, P], BF16, tag="w", name=f"w{i}") for i in range(2)]
                for half in range(2):
                    crange = range(half * 4, half * 4 + 4)
                    for mt in range(2):
                        psg = {}
                        for kt in range(2):
                            for c in crange:
                                if kt == 0:
                                    psg[c] = psmm.tile([128, CW], F32, tag="mm", name=f"psg{c}")
                                nc.tensor.matmul(psg[c][:], wgv(kt, mt),
                                                 xn[kt][:, cs(c)], start=(kt == 0), stop=(kt == 1))
                        for c in crange:
                            nc.scalar.activation(g_bf[mt][:, cs(c)], psg[c][:], AF.Identity,
                                                 bias=gbv[mt], scale=1.0)
                    for mt in range(2):
                        psv = {}
                        for kt in range(2):
                            for c in crange:
                                if kt == 0:
                                    psv[c] = psmm.tile([128, CW], F32, tag="mm", name=f"psv{c}")
                                nc.tensor.matmul(psv[c][:], wvv(kt, mt),
                                                 xn[kt][:, cs(c)], start=(kt == 0), stop=(kt == 1))
                        for c in crange:
                            # y = (v + vb) * g, straight from PSUM
                            nc.vector.scalar_tensor_tensor(y_bf[mt][:, cs(c)], psv[c][:],
                                                           vbv[mt], g_bf[mt][:, cs(c)],
                                                           ALU.add, ALU.mult)
                    for c in crange:
                        psa = psmm.tile([128, CW], F32, tag="mm")
                        nc.tensor.matmul(psa[:], ones_bf[:], a8row[0:1, cs(c)], start=True, stop=True)
                        nc.scalar.activation(abc[:, cs(c)], psa[:], AF.Copy, scale=1.0)
                    hs = slice(half * HALF, (half + 1) * HALF)
                    for ct in range(2):
                        nc.vector.tensor_mul(wt[ct][:, hs], y_bf[ct][:, hs], abc[:, hs])
                        nc.vector.tensor_add(wt[ct][:, hs], wt[ct][:, hs], xn[ct][:, hs])
                d["V"] = wt

            def phase_proj(s):
                d = st[s]
                wt = d["V"]
                for half in range(2):
                    crange = range(half * 4, half * 4 + 4)
                    hs = slice(half * HALF, (half + 1) * HALF)
                    for mt in range(2):
                        psp = {}
                        for kt in range(2):
                            for c in crange:
                                if kt == 0:
                                    psp[c] = psmm.tile([128, CW], F32, tag="mm", name=f"psp{c}")
                                nc.tensor.matmul(psp[c][:], wpv(kt, mt),
                                                 wt[kt][:, cs(c)], start=(kt == 0), stop=(kt == 1))
                        ot = outp.tile([128, HALF], F32, tag="out")
                        for c in crange:
                            ls = slice((c - half * 4) * CW, (c - half * 4 + 1) * CW)
                            if mt == 0:
                                nc.scalar.activation(ot[:, ls], psp[c][:], AF.Identity,
                                                     bias=pbv[mt], scale=1.0)
                            else:
                                nc.vector.tensor_scalar_add(ot[:, ls], psp[c][:], pbv[mt])
                        nc.gpsimd.dma_start(out=out_d[s, mt * 128:(mt + 1) * 128, hs], in_=ot[:])

            # interleave phases across the two samples for engine overlap;
            # keep the tiny softmax matmuls and the abc->w->V chain off the
            # critical PE stretches
            phase_load_norm(0)
            load_big_weights()
            phase_z(0)
            phase_softmax(0)
            phase_load_norm(1)
            phase_gvw(0)
            phase_z(1)
            phase_softmax(1)
            phase_proj(0)
            phase_gvw(1)
            phase_proj(1)

    return nc


def _prep_shared(inputs):
    import ml_dtypes
    bf16 = ml_dtypes.bfloat16
    f32 = np.float32
    gate_W = inputs["gate_W"].astype(f32)
    value_W = inputs["value_W"].astype(f32)
    z_W = inputs["z_W"].astype(f32)
    proj_W = inputs["proj_W"].astype(f32)
    gamma = inputs["gamma"].astype(f32)
    beta = inputs["beta"].astype(f32)
    zb = inputs["z_b"].astype(f32)

    inv_s = 1.0 / np.sqrt(np.float32(S))
    a = (gamma[0] * gamma[1]) * inv_s                       # [S]
    bv = (gamma[0] * beta[1] + gamma[1] * beta[0]) * inv_s  # [S]
    c0 = float(np.dot(beta[0], beta[1]) * inv_s + np.dot(bv, zb))
    bw = z_W.T @ bv                                         # [C]

    def kparts(wT):  # [C, M] -> [128, 2, M]
        return np.ascontiguousarray(wT.reshape(2, 128, -1).transpose(1, 0, 2))

    oha = np.zeros((128, NCHUNK, NCHUNK), f32)
    for c in range(NCHUNK):
        oha[:S, c, c] = a
    ohb = np.zeros((128, 2, NCHUNK, NCHUNK), f32)
    for kt in range(2):
        for c in range(NCHUNK):
            ohb[:, kt, c, c] = bw[kt * 128:(kt + 1) * 128]

    cbf = np.concatenate([
        kparts(z_W.T).reshape(128, 256),
        oha.reshape(128, 64),
        ohb.reshape(128, 128),
    ], axis=1).astype(bf16)                                 # [128, 448]

    cf32 = np.zeros((128, 8), f32)
    cf32[:, 0] = zb
    cf32[:, 1:3] = inputs["gate_b"].astype(f32).reshape(2, 128).T
    cf32[:, 3:5] = inputs["value_b"].astype(f32).reshape(2, 128).T
    cf32[:, 5:7] = inputs["proj_b"].astype(f32).reshape(2, 128).T
    cf32[:, 7] = c0

    wbig = np.concatenate([
        kparts(gate_W.T), kparts(value_W.T), kparts(proj_W.T)
    ], axis=2).astype(bf16)                                 # [128, 2, 768]

    return {"cbf": cbf, "cf32": cf32, "wbig": wbig}


def kernel(**inputs):
    if "nc" not in _cache:
        _cache["nc"] = _build()
    nc = _cache["nc"]

    shared = _prep_shared(inputs)
    x = inputs["x"].astype(np.float32).reshape(B, C, P)
    in_maps = []
    for core in range(NCORES):
        m = dict(shared)
        m["x"] = np.ascontiguousarray(x[core * SPC:(core + 1) * SPC])
        in_maps.append(m)

    res = run_bass_kernel_spmd(nc, in_maps, core_ids=list(range(NCORES)))
    out = np.concatenate([r["out"] for r in res.results], axis=0)
    A = np.concatenate([r["A"] for r in res.results], axis=0)
    return out.reshape(B, O, H, W), A.reshape(B, H, W)


# revision 19
# speedup vs baseline: 1.0065x; 1.0065x over previous
"""GAU-style module (InstanceNorm + gated spatial-softmax attention) on 8 trn2 cores.

Math notes (vs the PyTorch/JAX reference):
- 2D RoPE rotates q and k by the SAME per-(pair,pixel) angle, and sim is the
  per-pixel dot product, so the rotations cancel exactly:
      sim = q.k = sum_c (g0*z+b0)(g1*z+b1)
          = sum_c a_c z_c^2 + bv_c z_c + const,  a = g0*g1, bv = g0*b1+g1*b0
- The linear term is linear in x_n:  sum_c bv_c z_c = (z_W^T bv) . x_n + bv.z_b
  so z itself is only needed squared; Square() is fused into the PSUM read.
- Softmax over 4096 pixels/sample skips the max subtraction (|sim| stays far
  below fp32 exp overflow for this problem family; verified in testing).

Sharding: pure data parallel, 16 samples -> 2 per core, params replicated.

Implementation notes:
- All matmul operands are bf16 (1 cycle/row on the PE); PSUM accumulates fp32.
- Per-pixel softmax weights are applied via a rank-1 ones-matmul broadcast.
- (v + vb) * g is fused into one DVE scalar_tensor_tensor reading PSUM.
- dma_start dispatch costs ~0.65us of sequencer time each, so constants are
  packed into three tensors and output stores are batched per [128, 2048].
"""

import numpy as np

import concourse.bass as bass
import concourse.tile as tile
from concourse import mybir
from concourse.bass_utils import run_bass_kernel_spmd

F32 = mybir.dt.float32
BF16 = mybir.dt.bfloat16
AF = mybir.ActivationFunctionType
ALU = mybir.AluOpType

B, C, H, W, S, O = 16, 256, 64, 64, 128, 256
P = H * W            # 4096 pixels per sample
NCORES = 8
SPC = B // NCORES    # samples per core
NCHUNK = 8           # pixel chunks per sample
CW = P // NCHUNK     # chunk width = 512 (one fp32 PSUM bank)
HALF = P // 2
EPS = 1e-5

_cache: dict = {}

# The walrus build in this container refuses instructions carrying more than
# one semaphore wait ("Too many sync wait commands").  Tile freely emits 3-5
# waits per instruction (and ~27 on the tail drain).  Since engine queues
# execute in order, a wait carried by a same-engine NoOp immediately before
# the instruction is semantically identical — so split excess waits onto NoOp
# carriers at instruction-commit time.
MAXW = 1
_patched = False


def _install_tile_patches():
    global _patched
    if _patched:
        return
    _patched = True
    from concourse.vector_clock import ScopedClock

    orig_add = tile.TileContext._add_instruction

    def patched_add(self, inst):
        si = inst.sync_info
        if si is not None and si.on_wait and len(si.on_wait) > MAXW:
            waits = list(si.on_wait)
            excess, keep = waits[:-MAXW], waits[-MAXW:]
            for j, w in enumerate(excess):
                nop = mybir.InstNoOp(name=f"{inst.name}_ws{j}", ins=[], outs=[])
                nop.engine = inst.engine
                nop.sync_info = mybir.SyncInfo(on_wait=[w], on_update=[])
                orig_add(self, nop)
            inst.sync_info = mybir.SyncInfo(on_wait=keep, on_update=si.on_update or [])
        orig_add(self, inst)

    tile.TileContext._add_instruction = patched_add

    def patched_dab(self, tick_clock, wait_clock):
        drain_inst = self.nc.sync.drain()
        wait_clock.add_sem_waits(
            drain_inst.ins, ScopedClock({None: tick_clock.global_clock})
        )
        mi = drain_inst.ins
        si = mi.sync_info
        if si is not None and si.on_wait and len(si.on_wait) > MAXW:
            waits = list(si.on_wait)
            keep, rest = waits[:MAXW], waits[MAXW:]
            mi.sync_info = mybir.SyncInfo(on_wait=keep, on_update=si.on_update or [])
            for w in rest:
                n = self.nc.sync.nop()
                n.ins.sync_info = mybir.SyncInfo(on_wait=[w], on_update=[])
        self.nc.all_engine_barrier()
        popped = self.nc._tile_sem_poison_stack.pop()
        assert popped is self._sem_poison
        self.nc.clear_and_free_semaphores(list(self.sems.allocated().values()))
        self.nc.all_engine_barrier()

    tile.TileContext._drain_and_barrier = patched_dab


def _build():
    _install_tile_patches()
    nc = bass.Bass("TRN2", target_bir_lowering=False, debug=False)

    x_d = nc.dram_tensor("x", [SPC, C, P], F32, kind="ExternalInput")
    # bf16 const pack: [wz(2x128) | ohA(64) | ohBW(2x64)] = 448 cols
    cbf_d = nc.dram_tensor("cbf", [128, 448], BF16, kind="ExternalInput")
    # f32 const pack: [zb | gb0 gb1 | vb0 vb1 | pb0 pb1 | c0] = 8 cols
    cf32_d = nc.dram_tensor("cf32", [128, 8], F32, kind="ExternalInput")
    # big weights: [wg(2x256) | wv(2x256) | wp(2x256)] along the last dim
    wbig_d = nc.dram_tensor("wbig", [128, 2, 768], BF16, kind="ExternalInput")

    out_d = nc.dram_tensor("out", [SPC, O, P], F32, kind="ExternalOutput")
    a_d = nc.dram_tensor("A", [SPC, P], F32, kind="ExternalOutput")

    with tile.TileContext(nc) as tc:
        with (
            tc.tile_pool(name="consts", bufs=1) as consts,
            tc.tile_pool(name="xp", bufs=2) as xp,
            tc.tile_pool(name="xnp", bufs=4) as xnp,
            tc.tile_pool(name="gbf", bufs=2) as gbfp,
            tc.tile_pool(name="ybf", bufs=2) as ybfp,
            tc.tile_pool(name="abcp", bufs=2) as abcp,
            tc.tile_pool(name="wp", bufs=4) as wpool,
            tc.tile_pool(name="sqp", bufs=4) as sqp,
            tc.tile_pool(name="outp", bufs=3) as outp,
            tc.tile_pool(name="tiny", bufs=2) as tiny,
            tc.tile_pool(name="rowp", bufs=1) as rowp,
            tc.tile_pool(name="psmm", bufs=7, space="PSUM") as psmm,
            tc.tile_pool(name="psaux", bufs=1, space="PSUM") as psaux,
        ):
            # ---- constants / weights (loaded once, few big DMAs) ----
            cbf = consts.tile([128, 448], BF16, tag="cbf")
            cf32 = consts.tile([128, 8], F32, tag="cf32")
            wbig = consts.tile([128, 2, 768], BF16, tag="wbig")
            nc.sync.dma_start(out=cbf[:], in_=cbf_d[:])
            nc.sync.dma_start(out=cf32[:], in_=cf32_d[:])

            def load_big_weights():
                nc.sync.dma_start(out=wbig[:], in_=wbig_d[:])

            def wzv(kt):
                return cbf[:, kt * 128:(kt + 1) * 128]

            def ohav(c):
                return cbf[:, 256 + c * 8:256 + (c + 1) * 8]

            def ohbv(kt, c):
                return cbf[:, 320 + kt * 64 + c * 8:320 + kt * 64 + (c + 1) * 8]

            def wgv(kt, mt):
                return wbig[:, kt, mt * 128:(mt + 1) * 128]

            def wvv(kt, mt):
                return wbig[:, kt, 256 + mt * 128:256 + (mt + 1) * 128]

            def wpv(kt, mt):
                return wbig[:, kt, 512 + mt * 128:512 + (mt + 1) * 128]

            zb = cf32[:, 0:1]
            gbv = [cf32[:, 1:2], cf32[:, 2:3]]
            vbv = [cf32[:, 3:4], cf32[:, 4:5]]
            pbv = [cf32[:, 5:6], cf32[:, 6:7]]
            c0 = cf32[0:NCHUNK, 7:8]

            ones_bf = consts.tile([1, 128], BF16, tag="ones_bf")
            nc.vector.memset(ones_bf[:], 1.0)
            ones_f = consts.tile([1, 128], F32, tag="ones_f")
            nc.vector.memset(ones_f[:], 1.0)
            ones8 = consts.tile([NCHUNK, 1], F32, tag="ones8")
            nc.vector.memset(ones8[:], 1.0)
            eps_t = consts.tile([128, 1], F32, tag="eps")
            nc.vector.memset(eps_t[:], EPS)

            def cs(c):
                return slice(c * CW, (c + 1) * CW)

            # per-sample state carried between phases
            st: list[dict] = [dict() for _ in range(SPC)]

            def phase_load_norm(s):
                d = st[s]
                xn = []
                for ct in range(2):
                    xt = xp.tile([128, P], F32, tag="x", name=f"x{ct}")
                    for q in range(2):
                        qs = slice(q * HALF, (q + 1) * HALF)
                        nc.sync.dma_start(out=xt[:, qs],
                                          in_=x_d[s, ct * 128:(ct + 1) * 128, qs])
                    stt = tiny.tile([128, NCHUNK, 6], F32, tag="bnst")
                    for j in range(NCHUNK):
                        nc.vector.bn_stats(out=stt[:, j, :], in_=xt[:, cs(j)])
                    mv = tiny.tile([128, 2], F32, tag="mv")
                    nc.vector.bn_aggr(out=mv[:], in_=stt[:])
                    sd = tiny.tile([128, 1], F32, tag="sd")
                    nc.scalar.activation(sd[:], mv[:, 1:2], AF.Sqrt, bias=eps_t[:], scale=1.0)
                    rs_ = tiny.tile([128, 1], F32, tag="rstd")
                    nc.vector.reciprocal(rs_[:], sd[:])
                    nmr = tiny.tile([128, 1], F32, tag="nmr")
                    nc.vector.tensor_scalar(nmr[:], mv[:, 0:1], rs_[:], -1.0, ALU.mult, ALU.mult)
                    xnt = xnp.tile([128, P], BF16, tag="xn", name=f"xn{ct}")
                    for h in range(4):
                        hs = slice(h * (P // 4), (h + 1) * (P // 4))
                        if ct == 0:
                            nc.gpsimd.tensor_scalar(xnt[:, hs], xt[:, hs], rs_[:], nmr[:],
                                                    ALU.mult, ALU.add)
                        else:
                            nc.scalar.activation(xnt[:, hs], xt[:, hs], AF.Identity,
                                                 bias=nmr[:], scale=rs_[:])
                    xn.append(xnt)
                d["xn"] = xn

            def phase_z(s):
                d = st[s]
                xn = d["xn"]
                # z matmuls -> fused Square -> sim accumulation ([8, CW] psum)
                psim = psaux.tile([NCHUNK, CW], F32, tag="aux")
                for half in range(2):
                    crange = range(half * 4, half * 4 + 4)
                    psz = {}
                    for kt in range(2):
                        for c in crange:
                            if kt == 0:
                                psz[c] = psmm.tile([128, CW], F32, tag="mm", name=f"psz{c}")
                            nc.tensor.matmul(psz[c][:], wzv(kt), xn[kt][:, cs(c)],
                                             start=(kt == 0), stop=(kt == 1))
                    for c in crange:
                        sq = sqp.tile([128, CW], BF16, tag="sq")
                        nc.scalar.activation(sq[:], psz[c][:], AF.Square, bias=zb, scale=1.0)
                        nc.tensor.matmul(psim[:], ohav(c), sq[:],
                                         start=(c == 0), stop=False)
                        nc.tensor.matmul(psim[:], ohbv(0, c), xn[0][:, cs(c)],
                                         start=False, stop=False)
                        nc.tensor.matmul(psim[:], ohbv(1, c), xn[1][:, cs(c)],
                                         start=False, stop=(c == NCHUNK - 1))
                d["psim"] = psim

            def phase_softmax(s):
                d = st[s]
                psim = d.pop("psim")
                # softmax over the 4096 pixels (no max subtraction needed)
                e8 = tiny.tile([NCHUNK, CW], F32, tag="e8")
                rsum = tiny.tile([NCHUNK, 1], F32, tag="rsum")
                nc.scalar.activation(e8[:], psim[:], AF.Exp, bias=c0, scale=1.0,
                                     accum_out=rsum[:])
                pss = psaux.tile([1, 1], F32, tag="aux")
                nc.tensor.matmul(pss[:], ones8[:], rsum[:], start=True, stop=True)
                s_sb = tiny.tile([1, 1], F32, tag="ssb")
                nc.vector.tensor_copy(s_sb[:], pss[:])
                psbc = psaux.tile([128, 1], F32, tag="aux")
                nc.tensor.matmul(psbc[:], ones_f[:], s_sb[:], start=True, stop=True)
                rs_all = tiny.tile([128, 1], F32, tag="rsall")
                nc.vector.reciprocal(rs_all[:], psbc[:])
                a8 = tiny.tile([NCHUNK, CW], F32, tag="a8")
                nc.vector.tensor_scalar_mul(a8[:], e8[:], rs_all[0:NCHUNK, :])
                nc.gpsimd.dma_start(out=a_d[s].rearrange("(p f) -> p f", p=NCHUNK), in_=a8[:])
                a8b = tiny.tile([NCHUNK, CW], BF16, tag="a8b")
                nc.vector.tensor_copy(a8b[:], a8[:])
                # gather the 8 partition-rows into one [1, P] row so the
                # broadcast matmul's moving operand starts at partition 0
                a8row = rowp.tile([1, P], BF16, tag="a8row")
                nc.sync.dma_start(
                    out=a8row[0:1, :].rearrange("p (c f) -> p c f", c=NCHUNK),
                    in_=a8b[:, :])
                d["a8row"] = a8row

            def phase_gvw(s):
                d = st[s]
                xn, a8row = d["xn"], d["a8row"]
                g_bf = [gbfp.tile([128, P], BF16, tag="g", name=f"g{i}") for i in range(2)]
                y_bf = [ybfp.tile([128, P], BF16, tag="y", name=f"y{i}") for i in range(2)]
                abc = abcp.tile([128, P], BF16, tag="abc")
                wt = [wpool.tile([128, P], BF16, tag="w", name=f"w{i}") for i in range(2)]
                for half in range(2):
                    crange = range(half * 4, half * 4 + 4)
                    for mt in range(2):
                        psg = {}
                        for kt in range(2):
                            for c in crange:
                                if kt == 0:
                                    psg[c] = psmm.tile([128, CW], F32, tag="mm", name=f"psg{c}")
                                nc.tensor.matmul(psg[c][:], wgv(kt, mt),
                                                 xn[kt][:, cs(c)], start=(kt == 0), stop=(kt == 1))
                        for c in crange:
                            nc.scalar.activation(g_bf[mt][:, cs(c)], psg[c][:], AF.Identity,
                                                 bias=gbv[mt], scale=1.0)
                    for mt in range(2):
                        psv = {}
                        for kt in range(2):
                            for c in crange:
                                if kt == 0:
                                    psv[c] = psmm.tile([128, CW], F32, tag="mm", name=f"psv{c}")
                                nc.tensor.matmul(psv[c][:], wvv(kt, mt),
                                                 xn[kt][:, cs(c)], start=(kt == 0), stop=(kt == 1))
                        for c in crange:
                            # y = (v + vb) * g, straight from PSUM
                            nc.vector.scalar_tensor_tensor(y_bf[mt][:, cs(c)], psv[c][:],
                                                           vbv[mt], g_bf[mt][:, cs(c)],
                                                           ALU.add, ALU.mult)
                    for c in crange:
                        psa = psmm.tile([128, CW], F32, tag="mm")
                        nc.tensor.matmul(psa[:], ones_bf[:], a8row[0:1, cs(c)], start=True, stop=True)
                        nc.scalar.activation(abc[:, cs(c)], psa[:], AF.Copy, scale=1.0)
                    hs = slice(half * HALF, (half + 1) * HALF)
                    for ct in range(2):
                        nc.vector.tensor_mul(wt[ct][:, hs], y_bf[ct][:, hs], abc[:, hs])
                        nc.vector.tensor_add(wt[ct][:, hs], wt[ct][:, hs], xn[ct][:, hs])
                d["V"] = wt

            def phase_proj(s):
                d = st[s]
                wt = d["V"]
                for half in range(2):
                    crange = range(half * 4, half * 4 + 4)
                    hs = slice(half * HALF, (half + 1) * HALF)
                    for mt in range(2):
                        psp = {}
                        for kt in range(2):
                            for c in crange:
                                if kt == 0:
                                    psp[c] = psmm.tile([128, CW], F32, tag="mm", name=f"psp{c}")
                                nc.tensor.matmul(psp[c][:], wpv(kt, mt),
                                                 wt[kt][:, cs(c)], start=(kt == 0), stop=(kt == 1))
                        ot = outp.tile([128, HALF], F32, tag="out")
                        for c in crange:
                            ls = slice((c - half * 4) * CW, (c - half * 4 + 1) * CW)
                            if mt == 0:
                                nc.scalar.activation(ot[:, ls], psp[c][:], AF.Identity,
                                                     bias=pbv[mt], scale=1.0)
                            else:
                                nc.vector.tensor_scalar_add(ot[:, ls], psp[c][:], pbv[mt])
                        nc.sync.dma_start(out=out_d[s, mt * 128:(mt + 1) * 128, hs], in_=ot[:])

            # interleave phases across the two samples for engine overlap;
            # keep the tiny softmax matmuls and the abc->w->V chain off the
            # critical PE stretches
            phase_load_norm(0)
            load_big_weights()
            phase_z(0)
            phase_softmax(0)
            phase_load_norm(1)
            phase_gvw(0)
            phase_z(1)
            phase_softmax(1)
            phase_proj(0)
            phase_gvw(1)
            phase_proj(1)

    return nc


def _prep_shared(inputs):
    import ml_dtypes
    bf16 = ml_dtypes.bfloat16
    f32 = np.float32
    gate_W = inputs["gate_W"].astype(f32)
    value_W = inputs["value_W"].astype(f32)
    z_W = inputs["z_W"].astype(f32)
    proj_W = inputs["proj_W"].astype(f32)
    gamma = inputs["gamma"].astype(f32)
    beta = inputs["beta"].astype(f32)
    zb = inputs["z_b"].astype(f32)

    inv_s = 1.0 / np.sqrt(np.float32(S))
    a = (gamma[0] * gamma[1]) * inv_s                       # [S]
    bv = (gamma[0] * beta[1] + gamma[1] * beta[0]) * inv_s  # [S]
    c0 = float(np.dot(beta[0], beta[1]) * inv_s + np.dot(bv, zb))
    bw = z_W.T @ bv                                         # [C]

    def kparts(wT):  # [C, M] -> [128, 2, M]
        return np.ascontiguousarray(wT.reshape(2, 128, -1).transpose(1, 0, 2))

    oha = np.zeros((128, NCHUNK, NCHUNK), f32)
    for c in range(NCHUNK):
        oha[:S, c, c] = a
    ohb = np.zeros((128, 2, NCHUNK, NCHUNK), f32)
    for kt in range(2):
        for c in range(NCHUNK):
            ohb[:, kt, c, c] = bw[kt * 128:(kt + 1) * 128]

    cbf = np.concatenate([
        kparts(z_W.T).reshape(128, 256),
        oha.reshape(128, 64),
        ohb.reshape(128, 128),
    ], axis=1).astype(bf16)                                 # [128, 448]

    cf32 = np.zeros((128, 8), f32)
    cf32[:, 0] = zb
    cf32[:, 1:3] = inputs["gate_b"].astype(f32).reshape(2, 128).T
    cf32[:, 3:5] = inputs["value_b"].astype(f32).reshape(2, 128).T
    cf32[:, 5:7] = inputs["proj_b"].astype(f32).reshape(2, 128).T
    cf32[:, 7] = c0

    wbig = np.concatenate([
        kparts(gate_W.T), kparts(value_W.T), kparts(proj_W.T)
    ], axis=2).astype(bf16)                                 # [128, 2, 768]

    return {"cbf": cbf, "cf32": cf32, "wbig": wbig}


def kernel(**inputs):
    if "nc" not in _cache:
        _cache["nc"] = _build()
    nc = _cache["nc"]

    shared = _prep_shared(inputs)
    x = inputs["x"].astype(np.float32).reshape(B, C, P)
    in_maps = []
    for core in range(NCORES):
        m = dict(shared)
        m["x"] = np.ascontiguousarray(x[core * SPC:(core + 1) * SPC])
        in_maps.append(m)

    res = run_bass_kernel_spmd(nc, in_maps, core_ids=list(range(NCORES)))
    out = np.concatenate([r["out"] for r in res.results], axis=0)
    A = np.concatenate([r["A"] for r in res.results], axis=0)
    return out.reshape(B, O, H, W), A.reshape(B, H, W)


# revision 20
# speedup vs baseline: 1.0220x; 1.0154x over previous
"""GAU-style module (InstanceNorm + gated spatial-softmax attention) on 8 trn2 cores.

Math notes (vs the PyTorch/JAX reference):
- 2D RoPE rotates q and k by the SAME per-(pair,pixel) angle, and sim is the
  per-pixel dot product, so the rotations cancel exactly:
      sim = q.k = sum_c (g0*z+b0)(g1*z+b1)
          = sum_c a_c z_c^2 + bv_c z_c + const,  a = g0*g1, bv = g0*b1+g1*b0
- The linear term is linear in x_n:  sum_c bv_c z_c = (z_W^T bv) . x_n + bv.z_b
  so z itself is only needed squared; Square() is fused into the PSUM read.
- Softmax over 4096 pixels/sample skips the max subtraction (|sim| stays far
  below fp32 exp overflow for this problem family; verified in testing).

Sharding: pure data parallel, 16 samples -> 2 per core, params replicated.

Implementation notes:
- All matmul operands are bf16 (1 cycle/row on the PE); PSUM accumulates fp32.
- Per-pixel softmax weights are applied via a rank-1 ones-matmul broadcast.
- (v + vb) * g is fused into one DVE scalar_tensor_tensor reading PSUM.
- dma_start dispatch costs ~0.65us of sequencer time each, so constants are
  packed into three tensors and output stores are batched per [128, 2048].
"""

import numpy as np

import concourse.bass as bass
import concourse.tile as tile
from concourse import mybir
from concourse.bass_utils import run_bass_kernel_spmd

F32 = mybir.dt.float32
BF16 = mybir.dt.bfloat16
AF = mybir.ActivationFunctionType
ALU = mybir.AluOpType

B, C, H, W, S, O = 16, 256, 64, 64, 128, 256
P = H * W            # 4096 pixels per sample
NCORES = 8
SPC = B // NCORES    # samples per core
NCHUNK = 8           # pixel chunks per sample
CW = P // NCHUNK     # chunk width = 512 (one fp32 PSUM bank)
HALF = P // 2
EPS = 1e-5

_cache: dict = {}

# The walrus build in this container refuses instructions carrying more than
# one semaphore wait ("Too many sync wait commands").  Tile freely emits 3-5
# waits per instruction (and ~27 on the tail drain).  Since engine queues
# execute in order, a wait carried by a same-engine NoOp immediately before
# the instruction is semantically identical — so split excess waits onto NoOp
# carriers at instruction-commit time.
MAXW = 1
_patched = False


def _install_tile_patches():
    global _patched
    if _patched:
        return
    _patched = True
    from concourse.vector_clock import ScopedClock

    orig_add = tile.TileContext._add_instruction

    def patched_add(self, inst):
        si = inst.sync_info
        if si is not None and si.on_wait and len(si.on_wait) > MAXW:
            waits = list(si.on_wait)
            excess, keep = waits[:-MAXW], waits[-MAXW:]
            for j, w in enumerate(excess):
                nop = mybir.InstNoOp(name=f"{inst.name}_ws{j}", ins=[], outs=[])
                nop.engine = inst.engine
                nop.sync_info = mybir.SyncInfo(on_wait=[w], on_update=[])
                orig_add(self, nop)
            inst.sync_info = mybir.SyncInfo(on_wait=keep, on_update=si.on_update or [])
        orig_add(self, inst)

    tile.TileContext._add_instruction = patched_add

    def patched_dab(self, tick_clock, wait_clock):
        drain_inst = self.nc.sync.drain()
        wait_clock.add_sem_waits(
            drain_inst.ins, ScopedClock({None: tick_clock.global_clock})
        )
        mi = drain_inst.ins
        si = mi.sync_info
        if si is not None and si.on_wait and len(si.on_wait) > MAXW:
            waits = list(si.on_wait)
            keep, rest = waits[:MAXW], waits[MAXW:]
            mi.sync_info = mybir.SyncInfo(on_wait=keep, on_update=si.on_update or [])
            for w in rest:
                n = self.nc.sync.nop()
                n.ins.sync_info = mybir.SyncInfo(on_wait=[w], on_update=[])
        self.nc.all_engine_barrier()
        popped = self.nc._tile_sem_poison_stack.pop()
        assert popped is self._sem_poison
        self.nc.clear_and_free_semaphores(list(self.sems.allocated().values()))
        self.nc.all_engine_barrier()

    tile.TileContext._drain_and_barrier = patched_dab


def _build():
    _install_tile_patches()
    nc = bass.Bass("TRN2", target_bir_lowering=False, debug=False)

    x_d = nc.dram_tensor("x", [SPC, C, P], F32, kind="ExternalInput")
    # bf16 const pack: [wz(2x128) | ohA(64) | ohBW(2x64)] = 448 cols
    cbf_d = nc.dram_tensor("cbf", [128, 448], BF16, kind="ExternalInput")
    # f32 const pack: [zb | gb0 gb1 | vb0 vb1 | pb0 pb1 | c0] = 8 cols
    cf32_d = nc.dram_tensor("cf32", [128, 8], F32, kind="ExternalInput")
    # big weights: [wg(2x256) | wv(2x256) | wp(2x256)] along the last dim
    wbig_d = nc.dram_tensor("wbig", [128, 2, 768], BF16, kind="ExternalInput")

    out_d = nc.dram_tensor("out", [SPC, O, P], F32, kind="ExternalOutput")
    a_d = nc.dram_tensor("A", [SPC, P], F32, kind="ExternalOutput")

    with tile.TileContext(nc) as tc:
        with (
            tc.tile_pool(name="consts", bufs=1) as consts,
            tc.tile_pool(name="xp", bufs=2) as xp,
            tc.tile_pool(name="xnp", bufs=4) as xnp,
            tc.tile_pool(name="gbf", bufs=2) as gbfp,
            tc.tile_pool(name="ybf", bufs=2) as ybfp,
            tc.tile_pool(name="abcp", bufs=2) as abcp,
            tc.tile_pool(name="wp", bufs=4) as wpool,
            tc.tile_pool(name="sqp", bufs=4) as sqp,
            tc.tile_pool(name="outp", bufs=3) as outp,
            tc.tile_pool(name="tiny", bufs=2) as tiny,
            tc.tile_pool(name="rowp", bufs=1) as rowp,
            tc.tile_pool(name="psmm", bufs=7, space="PSUM") as psmm,
            tc.tile_pool(name="psaux", bufs=1, space="PSUM") as psaux,
        ):
            # ---- constants / weights (loaded once, few big DMAs) ----
            cbf = consts.tile([128, 448], BF16, tag="cbf")
            cf32 = consts.tile([128, 8], F32, tag="cf32")
            wbig = consts.tile([128, 2, 768], BF16, tag="wbig")
            nc.sync.dma_start(out=cbf[:], in_=cbf_d[:])
            nc.sync.dma_start(out=cf32[:], in_=cf32_d[:])

            def load_big_weights():
                nc.sync.dma_start(out=wbig[:], in_=wbig_d[:])

            def wzv(kt):
                return cbf[:, kt * 128:(kt + 1) * 128]

            def ohav(c):
                return cbf[:, 256 + c * 8:256 + (c + 1) * 8]

            def ohbv(kt, c):
                return cbf[:, 320 + kt * 64 + c * 8:320 + kt * 64 + (c + 1) * 8]

            def wgv(kt, mt):
                return wbig[:, kt, mt * 128:(mt + 1) * 128]

            def wvv(kt, mt):
                return wbig[:, kt, 256 + mt * 128:256 + (mt + 1) * 128]

            def wpv(kt, mt):
                return wbig[:, kt, 512 + mt * 128:512 + (mt + 1) * 128]

            zb = cf32[:, 0:1]
            gbv = [cf32[:, 1:2], cf32[:, 2:3]]
            vbv = [cf32[:, 3:4], cf32[:, 4:5]]
            pbv = [cf32[:, 5:6], cf32[:, 6:7]]
            c0 = cf32[0:NCHUNK, 7:8]

            ones_bf = consts.tile([1, 128], BF16, tag="ones_bf")
            nc.vector.memset(ones_bf[:], 1.0)
            ones_f = consts.tile([1, 128], F32, tag="ones_f")
            nc.vector.memset(ones_f[:], 1.0)
            ones8 = consts.tile([NCHUNK, 1], F32, tag="ones8")
            nc.vector.memset(ones8[:], 1.0)
            eps_t = consts.tile([128, 1], F32, tag="eps")
            nc.vector.memset(eps_t[:], EPS)

            def cs(c):
                return slice(c * CW, (c + 1) * CW)

            # per-sample state carried between phases
            st: list[dict] = [dict() for _ in range(SPC)]

            def phase_load_norm(s):
                d = st[s]
                xn = []
                for ct in range(2):
                    xt = xp.tile([128, P], F32, tag="x", name=f"x{ct}")
                    for q in range(2):
                        qs = slice(q * HALF, (q + 1) * HALF)
                        nc.sync.dma_start(out=xt[:, qs],
                                          in_=x_d[s, ct * 128:(ct + 1) * 128, qs])
                    stt = tiny.tile([128, NCHUNK, 6], F32, tag="bnst")
                    for j in range(NCHUNK):
                        nc.vector.bn_stats(out=stt[:, j, :], in_=xt[:, cs(j)])
                    mv = tiny.tile([128, 2], F32, tag="mv")
                    nc.vector.bn_aggr(out=mv[:], in_=stt[:])
                    sd = tiny.tile([128, 1], F32, tag="sd")
                    nc.scalar.activation(sd[:], mv[:, 1:2], AF.Sqrt, bias=eps_t[:], scale=1.0)
                    rs_ = tiny.tile([128, 1], F32, tag="rstd")
                    nc.vector.reciprocal(rs_[:], sd[:])
                    nmr = tiny.tile([128, 1], F32, tag="nmr")
                    nc.vector.tensor_scalar(nmr[:], mv[:, 0:1], rs_[:], -1.0, ALU.mult, ALU.mult)
                    xnt = xnp.tile([128, P], BF16, tag="xn", name=f"xn{ct}")
                    for h in range(4):
                        hs = slice(h * (P // 4), (h + 1) * (P // 4))
                        if ct == 0:
                            nc.gpsimd.tensor_scalar(xnt[:, hs], xt[:, hs], rs_[:], nmr[:],
                                                    ALU.mult, ALU.add)
                        else:
                            nc.scalar.activation(xnt[:, hs], xt[:, hs], AF.Identity,
                                                 bias=nmr[:], scale=rs_[:])
                    xn.append(xnt)
                d["xn"] = xn

            def phase_z(s):
                d = st[s]
                xn = d["xn"]
                # z matmuls -> fused Square -> sim accumulation ([8, CW] psum)
                psim = psaux.tile([NCHUNK, CW], F32, tag="aux")
                for half in range(2):
                    crange = range(half * 4, half * 4 + 4)
                    psz = {}
                    for kt in range(2):
                        for c in crange:
                            if kt == 0:
                                psz[c] = psmm.tile([128, CW], F32, tag="mm", name=f"psz{c}")
                            nc.tensor.matmul(psz[c][:], wzv(kt), xn[kt][:, cs(c)],
                                             start=(kt == 0), stop=(kt == 1))
                    for c in crange:
                        sq = sqp.tile([128, CW], BF16, tag="sq")
                        nc.scalar.activation(sq[:], psz[c][:], AF.Square, bias=zb, scale=1.0)
                        nc.tensor.matmul(psim[:], ohav(c), sq[:],
                                         start=(c == 0), stop=False)
                        nc.tensor.matmul(psim[:], ohbv(0, c), xn[0][:, cs(c)],
                                         start=False, stop=False)
                        nc.tensor.matmul(psim[:], ohbv(1, c), xn[1][:, cs(c)],
                                         start=False, stop=(c == NCHUNK - 1))
                d["psim"] = psim

            def phase_softmax(s):
                d = st[s]
                psim = d.pop("psim")
                # softmax over the 4096 pixels (no max subtraction needed)
                e8 = tiny.tile([NCHUNK, CW], F32, tag="e8")
                rsum = tiny.tile([NCHUNK, 1], F32, tag="rsum")
                nc.scalar.activation(e8[:], psim[:], AF.Exp, bias=c0, scale=1.0,
                                     accum_out=rsum[:])
                pss = psaux.tile([1, 1], F32, tag="aux")
                nc.tensor.matmul(pss[:], ones8[:], rsum[:], start=True, stop=True)
                s_sb = tiny.tile([1, 1], F32, tag="ssb")
                nc.vector.tensor_copy(s_sb[:], pss[:])
                psbc = psaux.tile([128, 1], F32, tag="aux")
                nc.tensor.matmul(psbc[:], ones_f[:], s_sb[:], start=True, stop=True)
                rs_all = tiny.tile([128, 1], F32, tag="rsall")
                nc.vector.reciprocal(rs_all[:], psbc[:])
                a8 = tiny.tile([NCHUNK, CW], F32, tag="a8")
                nc.vector.tensor_scalar_mul(a8[:], e8[:], rs_all[0:NCHUNK, :])
                nc.gpsimd.dma_start(out=a_d[s].rearrange("(p f) -> p f", p=NCHUNK), in_=a8[:])
                a8b = tiny.tile([NCHUNK, CW], BF16, tag="a8b")
                nc.vector.tensor_copy(a8b[:], a8[:])
                # gather the 8 partition-rows into one [1, P] row so the
                # broadcast matmul's moving operand starts at partition 0
                a8row = rowp.tile([1, P], BF16, tag="a8row")
                nc.sync.dma_start(
                    out=a8row[0:1, :].rearrange("p (c f) -> p c f", c=NCHUNK),
                    in_=a8b[:, :])
                d["a8row"] = a8row

            def phase_gvw(s):
                d = st[s]
                xn, a8row = d["xn"], d["a8row"]
                g_bf = [gbfp.tile([128, P], BF16, tag="g", name=f"g{i}") for i in range(2)]
                y_bf = [ybfp.tile([128, P], BF16, tag="y", name=f"y{i}") for i in range(2)]
                abc = abcp.tile([128, P], BF16, tag="abc")
                wt = [wpool.tile([128, P], BF16, tag="w", name=f"w{i}") for i in range(2)]
                for half in range(2):
                    crange = range(half * 4, half * 4 + 4)
                    for mt in range(2):
                        psg = {}
                        for kt in range(2):
                            for c in crange:
                                if kt == 0:
                                    psg[c] = psmm.tile([128, CW], F32, tag="mm", name=f"psg{c}")
                                nc.tensor.matmul(psg[c][:], wgv(kt, mt),
                                                 xn[kt][:, cs(c)], start=(kt == 0), stop=(kt == 1))
                        for c in crange:
                            nc.scalar.activation(g_bf[mt][:, cs(c)], psg[c][:], AF.Identity,
                                                 bias=gbv[mt], scale=1.0)
                    for mt in range(2):
                        psv = {}
                        for kt in range(2):
                            for c in crange:
                                if kt == 0:
                                    psv[c] = psmm.tile([128, CW], F32, tag="mm", name=f"psv{c}")
                                nc.tensor.matmul(psv[c][:], wvv(kt, mt),
                                                 xn[kt][:, cs(c)], start=(kt == 0), stop=(kt == 1))
                        for c in crange:
                            # y = (v + vb) * g, straight from PSUM
                            nc.vector.scalar_tensor_tensor(y_bf[mt][:, cs(c)], psv[c][:],
                                                           vbv[mt], g_bf[mt][:, cs(c)],
                                                           ALU.add, ALU.mult)
                    for c in crange:
                        psa = psmm.tile([128, CW], F32, tag="mm")
                        nc.tensor.matmul(psa[:], ones_bf[:], a8row[0:1, cs(c)], start=True, stop=True)
                        nc.scalar.activation(abc[:, cs(c)], psa[:], AF.Copy, scale=1.0)
                    hs = slice(half * HALF, (half + 1) * HALF)
                    for ct in range(2):
                        nc.vector.tensor_mul(wt[ct][:, hs], y_bf[ct][:, hs], abc[:, hs])
                        nc.vector.tensor_add(wt[ct][:, hs], wt[ct][:, hs], xn[ct][:, hs])
                d["V"] = wt

            def phase_proj(s):
                d = st[s]
                wt = d["V"]
                for half in range(2):
                    crange = list(range(half * 4, half * 4 + 4))
                    hs = slice(half * HALF, (half + 1) * HALF)
                    for mt in range(2):
                        psp = {}
                        for kt in range(2):
                            for c in crange:
                                if kt == 0:
                                    psp[c] = psmm.tile([128, CW], F32, tag="mm", name=f"psp{c}")
                                nc.tensor.matmul(psp[c][:], wpv(kt, mt),
                                                 wt[kt][:, cs(c)], start=(kt == 0), stop=(kt == 1))
                        ot = outp.tile([128, HALF], F32, tag="out")
                        for q in range(2):
                            for c in crange[q * 2:(q + 1) * 2]:
                                ls = slice((c - half * 4) * CW, (c - half * 4 + 1) * CW)
                                if mt == 0:
                                    nc.scalar.activation(ot[:, ls], psp[c][:], AF.Identity,
                                                         bias=pbv[mt], scale=1.0)
                                else:
                                    nc.vector.tensor_scalar_add(ot[:, ls], psp[c][:], pbv[mt])
                            qs_l = slice(q * 1024, (q + 1) * 1024)
                            qs_g = slice(half * HALF + q * 1024, half * HALF + (q + 1) * 1024)
                            nc.sync.dma_start(out=out_d[s, mt * 128:(mt + 1) * 128, qs_g],
                                              in_=ot[:, qs_l])

            # interleave phases across the two samples for engine overlap;
            # keep the tiny softmax matmuls and the abc->w->V chain off the
            # critical PE stretches
            phase_load_norm(0)
            load_big_weights()
            phase_z(0)
            phase_softmax(0)
            phase_load_norm(1)
            phase_gvw(0)
            phase_z(1)
            phase_softmax(1)
            phase_proj(0)
            phase_gvw(1)
            phase_proj(1)

    return nc


def _prep_shared(inputs):
    import ml_dtypes
    bf16 = ml_dtypes.bfloat16
    f32 = np.float32
    gate_W = inputs["gate_W"].astype(f32)
    value_W = inputs["value_W"].astype(f32)
    z_W = inputs["z_W"].astype(f32)
    proj_W = inputs["proj_W"].astype(f32)
    gamma = inputs["gamma"].astype(f32)
    beta = inputs["beta"].astype(f32)
    zb = inputs["z_b"].astype(f32)

    inv_s = 1.0 / np.sqrt(np.float32(S))
    a = (gamma[0] * gamma[1]) * inv_s                       # [S]
    bv = (gamma[0] * beta[1] + gamma[1] * beta[0]) * inv_s  # [S]
    c0 = float(np.dot(beta[0], beta[1]) * inv_s + np.dot(bv, zb))
    bw = z_W.T @ bv                                         # [C]

    def kparts(wT):  # [C, M] -> [128, 2, M]
        return np.ascontiguousarray(wT.reshape(2, 128, -1).transpose(1, 0, 2))

    oha = np.zeros((128, NCHUNK, NCHUNK), f32)
    for c in range(NCHUNK):
        oha[:S, c, c] = a
    ohb = np.zeros((128, 2, NCHUNK, NCHUNK), f32)
    for kt in range(2):
        for c in range(NCHUNK):
            ohb[:, kt, c, c] = bw[kt * 128:(kt + 1) * 128]

    cbf = np.concatenate([
        kparts(z_W.T).reshape(128, 256),
        oha.reshape(128, 64),
        ohb.reshape(128, 128),
    ], axis=1).astype(bf16)                                 # [128, 448]

    cf32 = np.zeros((128, 8), f32)
    cf32[:, 0] = zb
    cf32[:, 1:3] = inputs["gate_b"].astype(f32).reshape(2, 128).T
    cf32[:, 3:5] = inputs["value_b"].astype(f32).reshape(2, 128).T
    cf32[:, 5:7] = inputs["proj_b"].astype(f32).reshape(2, 128).T
    cf32[:, 7] = c0

    wbig = np.concatenate([
        kparts(gate_W.T), kparts(value_W.T), kparts(proj_W.T)
    ], axis=2).astype(bf16)                                 # [128, 2, 768]

    return {"cbf": cbf, "cf32": cf32, "wbig": wbig}


def kernel(**inputs):
    if "nc" not in _cache:
        _cache["nc"] = _build()
    nc = _cache["nc"]

    shared = _prep_shared(inputs)
    x = inputs["x"].astype(np.float32).reshape(B, C, P)
    in_maps = []
    for core in range(NCORES):
        m = dict(shared)
        m["x"] = np.ascontiguousarray(x[core * SPC:(core + 1) * SPC])
        in_maps.append(m)

    res = run_bass_kernel_spmd(nc, in_maps, core_ids=list(range(NCORES)))
    out = np.concatenate([r["out"] for r in res.results], axis=0)
    A = np.concatenate([r["A"] for r in res.results], axis=0)
    return out.reshape(B, O, H, W), A.reshape(B, H, W)
